# revision 30
# baseline (speedup 1.0000x reference)
"""Angular prototypical loss on 8 TRN2 NeuronCores (Bass/Tile, SPMD).

kernel(**inputs): takes FULL inputs (embeddings [65536,256] f32, labels
[65536] i32, num_classes), shards the batch across the 8 cores, runs one
SPMD Bass kernel (AllReduce of per-class prototype sums on-chip), returns
the scalar mean loss.

Per-core algorithm (rows = 8192 = 64 tiles of 128):
  Phase A: cast x->bf16 (DVE); row norm^2 on ScalarE (Square + ACT
    accumulator); invn = exp(-0.5 ln nsq) (stays in the exp/ln ACT table
    set, no sqrt table switch); one-hot scaled by invn via one dual-op
    tensor_scalar; S^T += x^T @ oh via 4 N=512 matmuls/tile (2 LDWEIGHTS,
    d-chunk outer).
  Transition: AllReduce S^T (bf16); xbar-transpose to class-major;
    per-class norm via fused affine_mul_reduce + exp/ln rsqrt; transpose
    back to d-major for Phase B.
  Phase B: cos row-tile via 4 N=512 matmuls (d-chunk outer, 2 LDWEIGHTS);
    exp with per-partition scale=invn/T AP reads PSUM directly, row-sum
    via the ACT accumulator; exp(m/T) extracted from the exps tile by a
    per-partition [label, label+1) tensor_mask_reduce (dot_mode="mask"),
    or m = x . shat[label] via indirect-DMA gather + affine_mul_reduce
    (dot_mode="amr").
  Epilogue: batched [128, 64] margin/CE math, sqrt via exp(0.5 ln x).
"""
import numpy as np
from concourse.bass_utils import run_bass_kernel_spmd

import math

import concourse.bass as bass
import concourse.mybir as mybir
import concourse.tile as tile
import concourse.bacc as bacc

P = 128
D = 256
C = 1024
NCORES = 8
MARGIN = 0.2
INV_T = 10.0
T = 0.1
COS_M = math.cos(MARGIN)
SIN_M = math.sin(MARGIN)
TH = math.cos(math.pi - MARGIN)

f32 = mybir.dt.float32
bf16 = mybir.dt.bfloat16
fp16 = mybir.dt.float16
i32 = mybir.dt.int32

AF = mybir.ActivationFunctionType
OP = mybir.AluOpType


RSQRT_K = 0x5f3759df


def _newton_rsqrt(nc, scr_pool, y, x, iters=2):
    """y = x^(-1/2) on the DVE only (no ACT tables): Quake bit-trick seed
    + `iters` Newton steps.  x, y: [P, k] f32 SBUF APs (may not alias)."""
    i32 = mybir.dt.int32
    OP = mybir.AluOpType
    shp = list(x.shape)
    t = scr_pool.tile(shp, mybir.dt.float32, tag="nrt_t")
    # seed: yi = K - (xi >> 1)  (negate+add runs in the f32 ALU domain;
    # the ~2^6-LSB rounding it costs on the seed is absorbed by Newton)
    nc.vector.tensor_scalar(y.bitcast(i32), x.bitcast(i32), 1, None,
                            OP.logical_shift_right)
    nc.vector.tensor_scalar(y.bitcast(i32), y.bitcast(i32), -1, RSQRT_K,
                            OP.mult, OP.add)
    for _ in range(iters):
        nc.vector.tensor_tensor(t[:], x, y, op=OP.mult)
        nc.vector.tensor_tensor(t[:], t[:], y, op=OP.mult)
        nc.vector.tensor_scalar(t[:], t[:], -0.5, 1.5, OP.mult, OP.add)
        nc.vector.tensor_tensor(y, y, t[:], op=OP.mult)


def build(nt: int = 64, group: int = 8, dot_mode: str = "amr",
          gb: int = 1, psb: int = 3, ramp: bool = True,
          expap: bool = True, ohfold: bool = True):
    """nt: row-tiles per core (rows/core = 128*nt). group: tiles per DMA group."""
    BL = P * nt
    ng = nt // group
    assert nt % group == 0

    nc = bacc.Bacc("TRN2", target_bir_lowering=False, debug=False,
                   num_devices=NCORES)
    emb = nc.declare_dram_parameter("embeddings", [BL, D], f32, isOutput=False)
    lab = nc.declare_dram_parameter("labels", [BL], i32, isOutput=False)
    out = nc.declare_dram_parameter("out", [P, 1], f32, isOutput=True)

    emb_g = emb.ap().rearrange("(p q) d -> p q d", p=P)      # [128, nt, 256]
    lab_pn = lab.ap().rearrange("(p n) -> p n", p=P)         # [128, nt]

    with tile.TileContext(nc) as tc:
        with (
            tc.tile_pool(name="big", bufs=1) as big,
            tc.tile_pool(name="stage", bufs=2) as stage,
            tc.tile_pool(name="ohp", bufs=4) as ohp,
            tc.tile_pool(name="gat", bufs=8) as gat,
            tc.tile_pool(name="scr", bufs=4) as scr,
            tc.tile_pool(name="expp", bufs=2) as expp,
            tc.tile_pool(name="dram", bufs=1, space="DRAM") as dram,
        ):
            ar_in = dram.tile([P, 2 * C], bf16, tag="ar_in")
            ar_out = dram.tile([P, 2 * C], bf16, tag="ar_out",
                               addr_space="Shared")
            ar_in1 = dram.tile([P, 2 * C], bf16, tag="ar_in1")
            ar_out1 = dram.tile([P, 2 * C], bf16, tag="ar_out1",
                                addr_space="Shared")
            if dot_mode == "amr":
                shat_dram = dram.tile([C, D], bf16, tag="shat_dram")
                shat_dram_v = shat_dram.rearrange("(j p) (k r) -> p k j r",
                                                  p=P, k=2)

            # ---- persistent SBUF ----
            x_bf = big.tile([P, nt * D], bf16, tag="x_bf")
            xT = big.tile([P, nt, 2, P], bf16, tag="xT")
            sT = big.tile([P, 2, C], bf16, tag="sT")      # [d%128, dk, c]
            sC = big.tile([P, 2, 8, P], bf16, tag="sC")   # [c%128, dk, j, d%128]
            ones_col = big.tile([P, 1], bf16, tag="ones_col")
            ones_row = big.tile([1, P], bf16, tag="ones_row")
            plnt_row = big.tile([1, C], f32, tag="plnt_row")
            pinv_row = big.tile([1, C], bf16, tag="pinv_row")
            sg = big.tile([P, 2, C], bf16, tag="sg")
            s_loc = big.tile([P, 2, C], bf16, tag="s_loc")
            s_h2 = big.tile([P, 2, C], bf16, tag="s_h2")
            a1_sb = big.tile([P, 2 * C], bf16, tag="a1_sb")
            a2_sb = big.tile([P, 2 * C], bf16, tag="a2_sb")
            lab_i = big.tile([P, nt], i32, tag="lab_i")
            lab_f = big.tile([P, nt], f32, tag="lab_f")
            labp1 = big.tile([P, nt], f32, tag="labp1")
            nsq = big.tile([P, nt], f32, tag="nsq")
            lnt = big.tile([P, nt], f32, tag="lnt")
            invn = big.tile([P, nt], f32, tag="invn")
            invnT = big.tile([P, nt], f32, tag="invnT")
            m_raw = big.tile([P, nt], f32, tag="m_raw")
            sumexp = big.tile([P, nt], f32, tag="sumexp")
            iota16 = big.tile([P, C], fp16, tag="iota16")

            nc.vector.memset(ones_col[:], 1.0)
            nc.vector.memset(ones_row[:], 1.0)
            nc.gpsimd.iota(iota16[:], pattern=[[1, C]], base=0,
                           channel_multiplier=0,
                           allow_small_or_imprecise_dtypes=True)
            nc.sync.dma_start(out=lab_i[:], in_=lab_pn)
            nc.vector.tensor_copy(lab_f[:], lab_i[:])
            nc.vector.tensor_scalar(labp1[:], lab_f[:], 1.0, None, OP.add)

            # ================= Phase A =================
            half = nt // 2
            with tc.tile_pool(name="psA", bufs=1, space="PSUM") as psA:
                sacc = [[[psA.tile([P, 512], f32, tag=f"sacc{h}{dk}{ch}",
                                   name=f"sacc{h}{dk}{ch}")
                          for ch in range(2)] for dk in range(2)]
                        for h in range(2)]
                gsizes = []
                rem = nt
                for sz in ((2, 2, 4) if ramp else ()):
                    if rem > group:
                        gsizes.append(sz)
                        rem -= sz
                while rem > 0:
                    gsizes.append(min(group, rem))
                    rem -= min(group, rem)
                gstart = [sum(gsizes[:i]) for i in range(len(gsizes))]
                for g, (g0, gsz) in enumerate(zip(gstart, gsizes)):
                    raw = stage.tile([P, group, D], f32, tag="raw")
                    nc.sync.dma_start(out=raw[:, :gsz, :],
                                      in_=emb_g[:, g0:g0 + gsz, :])
                    gsl = slice(g0, g0 + gsz)
                    for t in range(gsz):
                        n = g0 + t
                        x_n = x_bf[:, n * D:(n + 1) * D]
                        nc.vector.tensor_copy(x_n, raw[:, t, :])
                        sq = scr.tile([P, D], f32, tag="sq")
                        nc.scalar.activation(sq[:], raw[:, t, :], AF.Square,
                                             accum_out=nsq[:, n:n + 1])
                    # invn = nsq^(-1/2) on the DVE (no ACT table thrash)
                    _newton_rsqrt(nc, scr, invn[:, gsl], nsq[:, gsl])
                    nc.vector.tensor_scalar_mul(invnT[:, gsl], invn[:, gsl],
                                                INV_T)
                    for t in range(gsz):
                        n = g0 + t
                        oh = ohp.tile([P, C], bf16, tag="oh")
                        if ohfold:
                            nc.vector.tensor_scalar(
                                oh[:], iota16[:], lab_f[:, n:n + 1],
                                invn[:, n:n + 1], OP.is_equal, OP.mult)
                        else:
                            nc.vector.tensor_scalar(
                                oh[:], iota16[:], lab_f[:, n:n + 1],
                                None, OP.is_equal)
                        h = n // half
                        for dk in range(2):
                            lhsT = x_bf[:, n * D + dk * P:n * D + (dk + 1) * P]
                            for ch in range(2):
                                nc.tensor.matmul(
                                    out=sacc[h][dk][ch][:],
                                    lhsT=lhsT,
                                    rhs=oh[:, ch * 512:(ch + 1) * 512],
                                    start=(n % half == 0),
                                    stop=(n % half == half - 1))
                    nc.sync.dma_start_transpose(
                        out=xT[:, g0:g0 + gsz, :, :],
                        in_=x_bf[:, g0 * D:(g0 + gsz) * D])
                # local halves -> one AllReduce at the end
                for dk in range(2):
                    for ch in range(2):
                        nc.scalar.copy(s_loc[:, dk, ch * 512:(ch + 1) * 512],
                                       sacc[0][dk][ch][:])
                        nc.scalar.copy(s_h2[:, dk, ch * 512:(ch + 1) * 512],
                                       sacc[1][dk][ch][:])
            nc.vector.tensor_tensor(s_loc[:].rearrange("p k c -> p (k c)"),
                                    s_loc[:].rearrange("p k c -> p (k c)"),
                                    s_h2[:].rearrange("p k c -> p (k c)"),
                                    op=OP.add)
            nc.sync.dma_start(out=ar_in[:], in_=s_loc[:])
            nc.gpsimd.collective_compute(
                "AllReduce", OP.add,
                replica_groups=[list(range(NCORES))],
                ins=[ar_in[:].opt()], outs=[ar_out[:].opt()])
            nc.sync.dma_start(out=sg[:].rearrange("p k c -> p (k c)"),
                              in_=ar_out[:])

            # ---- normalize prototypes in-place in d-major layout ----
            # per-class norm^2 via ones-matmul (partition reduction on PE),
            # rsqrt on [1, C] via ACT exp/ln, broadcast via rank-1 matmul.
            sq = big.tile([P, 2, C], bf16, tag="sq")
            nc.vector.tensor_tensor(sq[:].rearrange("p k c -> p (k c)"),
                                    sg[:].rearrange("p k c -> p (k c)"),
                                    sg[:].rearrange("p k c -> p (k c)"),
                                    op=OP.mult)
            with tc.tile_pool(name="psT", bufs=1, space="PSUM") as psT:
                pnsq_ps = psT.tile([1, C], f32, tag="pnsq_ps")
                pinvB = psT.tile([P, C], f32, tag="pinvB")
                for ch in range(2):
                    for k in range(2):
                        nc.tensor.matmul(
                            out=pnsq_ps[:, ch * 512:(ch + 1) * 512],
                            lhsT=ones_col[:],
                            rhs=sq[:, k, ch * 512:(ch + 1) * 512],
                            start=(k == 0), stop=(k == 1))
                nc.scalar.activation(plnt_row[:], pnsq_ps[:], AF.Ln)
                nc.scalar.activation(pinv_row[:], plnt_row[:], AF.Exp,
                                     scale=-0.5)
                for ch in range(2):
                    nc.tensor.matmul(
                        out=pinvB[:, ch * 512:(ch + 1) * 512],
                        lhsT=ones_row[:],
                        rhs=pinv_row[:, ch * 512:(ch + 1) * 512],
                        start=True, stop=True)
                for k in range(2):
                    nc.vector.tensor_tensor(sT[:, k, :], sg[:, k, :],
                                            pinvB[:], op=OP.mult)
            if dot_mode == "amr":
                # class-major copy of shat to DRAM for the indirect gather
                # (off the critical path: only the m-dot consumes it)
                nc.sync.dma_start_transpose(
                    out=sC[:], in_=sT[:].rearrange("p k c -> p (k c)"))
                nc.sync.dma_start(out=shat_dram_v, in_=sC[:])

            # ================= Phase B =================
            with tc.tile_pool(name="psB", bufs=psb, space="PSUM") as psB:
                Gts = {}
                for n in range(nt):
                    if dot_mode == "amr" and n % gb == 0:
                        if gb == 1:
                            Gp = gat.tile([P, D], bf16, tag=f"G{n % 8}",
                                          name=f"G_{n}")
                            Gp1 = Gp.rearrange("p d -> p 1 d") if False else None
                        else:
                            Gp = gat.tile([P, gb, D], bf16,
                                          tag=f"G{(n // gb) % 4}",
                                          name=f"G_{n}")
                        nc.gpsimd.indirect_dma_start(
                            out=Gp[:], out_offset=None,
                            in_=shat_dram[:],
                            in_offset=bass.IndirectOffsetOnAxis(
                                ap=lab_i[:, n:n + gb], axis=0))
                        Gts[n] = Gp
                    cos_ps = psB.tile([P, C], f32, tag="cos")
                    for dk in range(2):
                        for ch in range(2):
                            nc.tensor.matmul(
                                out=cos_ps[:, ch * 512:(ch + 1) * 512],
                                lhsT=xT[:, n, dk, :],
                                rhs=sT[:, dk, ch * 512:(ch + 1) * 512],
                                start=(dk == 0), stop=(dk == 1))
                    exps = expp.tile([P, C], bf16, tag="exps")
                    nc.scalar.activation(
                        exps[:], cos_ps[:], AF.Exp,
                        scale=(invnT[:, n:n + 1] if expap else INV_T),
                        accum_out=sumexp[:, n:n + 1])
                    if dot_mode == "amr":
                        trm = scr.tile([P, D], bf16, tag="trm")
                        nc.vector.affine_mul_reduce(
                            out=trm[:], accum_out=m_raw[:, n:n + 1],
                            in0=x_bf[:, n * D:(n + 1) * D],
                            in1=(Gts[n][:] if gb == 1
                                 else Gts[n - n % gb][:, n % gb, :]),
                            scale=1.0, bias=0.0)
                    else:
                        # m_raw[p] = exp(m/T) = exps[p, label[p]]
                        trm = scr.tile([P, C], bf16, tag="trm")
                        nc.vector.tensor_mask_reduce(
                            out=trm[:], in_=exps[:],
                            mask_start=lab_f[:, n:n + 1],
                            mask_end=labp1[:, n:n + 1],
                            scale=1.0, accum_in=0.0, op=OP.max,
                            accum_out=m_raw[:, n:n + 1])

            # ================= epilogue (batched [P, nt]) ================
            m_all = big.tile([P, nt], f32, tag="m_all")
            expm = big.tile([P, nt], f32, tag="expm")
            b1 = big.tile([P, nt], f32, tag="b1")
            b2 = big.tile([P, nt], f32, tag="b2")
            b3 = big.tile([P, nt], f32, tag="b3")
            b4 = big.tile([P, nt], f32, tag="b4")
            mask = big.tile([P, nt], mybir.dt.uint8, tag="mask")
            phi_f = big.tile([P, nt], f32, tag="phi_f")

            if dot_mode == "amr":
                nc.vector.tensor_tensor(m_all[:], m_raw[:], invn[:],
                                        op=OP.mult)
                nc.scalar.activation(expm[:], m_all[:], AF.Exp, scale=INV_T)
            else:
                # m = T ln(exp(m/T));  expm = exp(m/T) directly
                nc.vector.tensor_copy(expm[:], m_raw[:])
                nc.scalar.activation(b1[:], m_raw[:], AF.Ln)
                nc.vector.tensor_scalar_mul(m_all[:], b1[:], T)
            nc.vector.tensor_tensor(b1[:], m_all[:], m_all[:], op=OP.mult)
            nc.vector.tensor_scalar(b1[:], b1[:], -1.0, 1.0, OP.mult, OP.add)
            nc.vector.tensor_scalar_max(b1[:], b1[:], 1e-30)
            # sin = sqrt(b1) = b1 * rsqrt(b1), Newton on the DVE
            _newton_rsqrt(nc, scr, b2[:], b1[:])
            nc.vector.tensor_tensor(b2[:], b2[:], b1[:], op=OP.mult)
            nc.vector.tensor_scalar_mul(b3[:], m_all[:], COS_M)
            nc.vector.tensor_scalar(b2[:], b2[:], -SIN_M, None, OP.mult)
            nc.vector.tensor_add(b3[:], b3[:], b2[:])           # phi
            nc.vector.tensor_scalar(mask[:], m_all[:], TH, None, OP.is_gt)
            nc.vector.tensor_scalar(b4[:], m_all[:], -MARGIN, None, OP.add)
            nc.vector.select(phi_f[:], mask[:], b3[:], b4[:])
            nc.scalar.activation(b2[:], phi_f[:], AF.Exp, scale=INV_T)
            nc.vector.tensor_sub(b1[:], sumexp[:], expm[:])
            nc.vector.tensor_add(b1[:], b1[:], b2[:])           # Z'
            nc.scalar.activation(b2[:], b1[:], AF.Ln)
            nc.vector.tensor_scalar_mul(b3[:], phi_f[:], INV_T)
            nc.vector.tensor_sub(b2[:], b2[:], b3[:])           # nll
            part = big.tile([P, 1], f32, tag="part")
            nc.vector.reduce_sum(part[:], b2[:], axis=mybir.AxisListType.X)
            nc.sync.dma_start(out=out[:], in_=part[:])

    nc.compile()
    return nc


_NC_CACHE = {}


def kernel(embeddings, labels, num_classes=None, **_ignored):
    embeddings = np.ascontiguousarray(embeddings, dtype=np.float32)
    labels = np.ascontiguousarray(labels, dtype=np.int32)
    B = embeddings.shape[0]
    BL = B // NCORES

    if "nc" not in _NC_CACHE:
        _NC_CACHE["nc"] = build()
    nc = _NC_CACHE["nc"]

    in_maps = [{"embeddings": embeddings[i * BL:(i + 1) * BL],
                "labels": labels[i * BL:(i + 1) * BL]}
               for i in range(NCORES)]
    res = run_bass_kernel_spmd(nc, in_maps, list(range(NCORES)))
    total = 0.0
    for i in range(NCORES):
        total += res.results[i]["out"].astype(np.float64).sum()
    return np.float32(total / B)
